# revision 1
# baseline (speedup 1.0000x reference)
"""Trainium2 Bass kernel for hyperedge segment-reduce (Maxmin) + MLP decoder.

Computation (matches the reference nn.Module):
    feats = v_feat[node_ids]                        # [E, D] gather
    emb   = segment_max(feats) - segment_min(feats) # [NH, D], segments = groups of 16
    out   = sigmoid(relu(relu(emb@W1+b1)@W2+b2)@W3+b3)   # [NH, 1]

Sharding: hyperedges are split evenly across 8 NeuronCores; v_feat and the
MLP weights are replicated.  No cross-device communication is needed.

Per-core device program:
  - indirect-DMA gather: members of hyperedge h land on partition (h mod 128),
    16 members contiguous in the free dim.
  - DVE halving-tree (4 levels of tensor_tensor max / min) computes the
    segment max and min; emb = max - min (hedge-on-partition layout).
  - PE transpose per 128x128 block -> emb^T (feature-on-partition layout).
  - 3-layer MLP on PE (matmuls) + ACT (bias+relu / bias+sigmoid).
"""

import os
import numpy as np

import concourse.bass as bass
import concourse.mybir as mybir
from concourse import bacc, tile, bass_utils
from concourse.masks import make_identity

# ---------------------------------------------------------------- constants
N_NODES = 100000
D = 128
NH = 50000
G = 16
E = NH * G
NCORES = 8
H_CORE = NH // NCORES           # 6250 hyperedges per core
BLKS = 49                       # ceil(6250/128)
HPAD = BLKS * 128               # 6272 (padded hyperedges per core)
CHUNK_BLKS = 7                  # hedge-blocks per gather chunk
NCHUNK = BLKS // CHUNK_BLKS     # 7 chunks
CH = CHUNK_BLKS * 128           # 896 hedges per chunk
MEMB = CH * G                   # 14336 gathered rows per chunk
NSPLIT = 448                    # matmul free-dim split (2 x 448 = 896)

# dtype knobs ("f32" or "bf16").  The gather is instruction-issue bound
# (not byte bound), so fp32 costs nothing extra and is exact.
GATHER_DT = os.environ.get("KERNEL_GATHER_DT", "f32")
MLP_DT = os.environ.get("KERNEL_MLP_DT", "f32")

_DT = {"f32": mybir.dt.float32, "bf16": mybir.dt.bfloat16}

f32 = mybir.dt.float32
i32 = mybir.dt.int32


# ---------------------------------------------------------------- device IR
def build_module():
    gdt = _DT[GATHER_DT]
    mdt = _DT[MLP_DT]

    nc = bacc.Bacc(
        "TRN2",
        target_bir_lowering=False,
        debug=False,
        enable_asserts=False,
        num_devices=NCORES,
    )

    vfeat = nc.dram_tensor("vfeat", [N_NODES, D], gdt, kind="ExternalInput")
    idx = nc.dram_tensor("idx", [128, BLKS * G], i32, kind="ExternalInput")
    w1 = nc.dram_tensor("w1", [128, 256], mdt, kind="ExternalInput")
    b1 = nc.dram_tensor("b1", [128, 2], f32, kind="ExternalInput")
    w2 = nc.dram_tensor("w2", [128, 256], mdt, kind="ExternalInput")
    b2 = nc.dram_tensor("b2", [128, 1], f32, kind="ExternalInput")
    w3 = nc.dram_tensor("w3", [128, 1], mdt, kind="ExternalInput")
    b3 = nc.dram_tensor("b3", [1, 1], f32, kind="ExternalInput")
    idn = nc.dram_tensor("ident", [128, 128], mdt, kind="ExternalInput")
    out = nc.dram_tensor("out", [HPAD], f32, kind="ExternalOutput")

    out2d = out.ap().rearrange("(a b) -> a b", a=1)  # [1, HPAD]

    with tile.TileContext(nc) as tc:
        with (
            tc.tile_pool(name="const", bufs=1) as cp,
            tc.tile_pool(name="mem", bufs=2) as mp,
            tc.tile_pool(name="scr", bufs=2) as sp,
            tc.tile_pool(name="mlp", bufs=2) as lp,
            tc.tile_pool(name="pst", bufs=2, space="PSUM") as pt,
            tc.tile_pool(name="psm", bufs=1, space="PSUM") as pm,
        ):
            ident = cp.tile([128, 128], mdt)
            w1_t = cp.tile([128, 256], mdt)
            b1_t = cp.tile([128, 2], f32)
            w2_t = cp.tile([128, 256], mdt)
            b2_t = cp.tile([128, 1], f32)
            w3_t = cp.tile([128, 1], mdt)
            b3_t = cp.tile([1, 1], f32)

            embT = None
            for blk in range(BLKS):
                # ---- per-block index load (sync engine, deep prefetch) ----
                ib = mp.tile([128, G], i32, tag="idxb", bufs=8)
                nc.sync.dma_start(out=ib[:], in_=idx.ap()[:, blk * G:(blk + 1) * G])

                # ---- gather 16 members for 128 hyperedges ----
                # HW indirect DMA only supports one row per partition per
                # instruction (idx [128,1], dest [128, D]).
                Mb = mp.tile([128, G * D], gdt, tag="mb", bufs=6)
                M4 = Mb[:].rearrange("p (m d) -> p m d", d=D)
                for m in range(G):
                    nc.gpsimd.indirect_dma_start(
                        out=M4[:, m, :],
                        out_offset=None,
                        in_=vfeat.ap(),
                        in_offset=bass.IndirectOffsetOnAxis(
                            ap=ib[:, m:m + 1], axis=0),
                    )

                if blk == 0:
                    nc.sync.dma_start(out=ident[:], in_=idn.ap())
                    nc.sync.dma_start(out=w1_t[:], in_=w1.ap())
                    nc.sync.dma_start(out=b1_t[:], in_=b1.ap())
                    nc.sync.dma_start(out=w2_t[:], in_=w2.ap())
                    nc.sync.dma_start(out=b2_t[:], in_=b2.ap())
                    nc.sync.dma_start(out=w3_t[:], in_=w3.ap())
                    nc.sync.dma_start(out=b3_t[:], in_=b3.ap())

                # ---- segment max (tree into scratch) ----
                mx = sp.tile([128, 8 * D], gdt, tag="mx")
                mx4 = mx[:].rearrange("p (m d) -> p m d", d=D)
                nc.vector.tensor_tensor(
                    out=mx4, in0=M4[:, 0:8, :], in1=M4[:, 8:16, :],
                    op=mybir.AluOpType.max)
                nc.vector.tensor_tensor(
                    out=mx4[:, 0:4, :], in0=mx4[:, 0:4, :],
                    in1=mx4[:, 4:8, :], op=mybir.AluOpType.max)
                nc.vector.tensor_tensor(
                    out=mx4[:, 0:2, :], in0=mx4[:, 0:2, :],
                    in1=mx4[:, 2:4, :], op=mybir.AluOpType.max)
                nc.vector.tensor_tensor(
                    out=mx4[:, 0:1, :], in0=mx4[:, 0:1, :],
                    in1=mx4[:, 1:2, :], op=mybir.AluOpType.max)

                # ---- segment min (tree in place inside Mb) ----
                nc.vector.tensor_tensor(
                    out=M4[:, 0:8, :], in0=M4[:, 0:8, :],
                    in1=M4[:, 8:16, :], op=mybir.AluOpType.min)
                nc.vector.tensor_tensor(
                    out=M4[:, 0:4, :], in0=M4[:, 0:4, :],
                    in1=M4[:, 4:8, :], op=mybir.AluOpType.min)
                nc.vector.tensor_tensor(
                    out=M4[:, 0:2, :], in0=M4[:, 0:2, :],
                    in1=M4[:, 2:4, :], op=mybir.AluOpType.min)
                nc.vector.tensor_tensor(
                    out=M4[:, 0:1, :], in0=M4[:, 0:1, :],
                    in1=M4[:, 1:2, :], op=mybir.AluOpType.min)

                # ---- emb = max - min  (hedge-on-partition layout) ----
                emb = sp.tile([128, D], mdt, tag="emb", bufs=3)
                emb3 = emb[:].rearrange("p (m d) -> p m d", m=1)
                nc.vector.tensor_tensor(
                    out=emb3, in0=mx4[:, 0:1, :], in1=M4[:, 0:1, :],
                    op=mybir.AluOpType.subtract)

                # ---- transpose into the 7-block group tile ----
                gi = blk % CHUNK_BLKS
                if gi == 0:
                    embT = lp.tile([128, CH], mdt, tag="embT")
                ptile = pt.tile([128, 128], mdt, tag="ptr")
                nc.tensor.transpose(
                    out=ptile[:], in_=emb[:], identity=ident[:])
                nc.scalar.copy(
                    out=embT[:, gi * 128:(gi + 1) * 128], in_=ptile[:])

                if gi != CHUNK_BLKS - 1:
                    continue
                ch = blk // CHUNK_BLKS

                # ---- MLP on the completed 896-hyperedge group ----
                for s in range(CH // NSPLIT):
                    ns = slice(s * NSPLIT, (s + 1) * NSPLIT)
                    h1 = lp.tile([128, 2 * NSPLIT], mdt, tag="h1")
                    p2 = pm.tile([128, NSPLIT], f32, tag="p2")
                    for o in range(2):
                        p1 = pm.tile([128, NSPLIT], f32, tag=f"p1{o}")
                        nc.tensor.matmul(
                            out=p1[:], lhsT=w1_t[:, o * 128:(o + 1) * 128],
                            rhs=embT[:, ns], start=True, stop=True)
                        nc.scalar.activation(
                            out=h1[:, o * NSPLIT:(o + 1) * NSPLIT], in_=p1[:],
                            func=mybir.ActivationFunctionType.Relu,
                            bias=b1_t[:, o:o + 1])
                    nc.tensor.matmul(
                        out=p2[:], lhsT=w2_t[:, 0:128], rhs=h1[:, 0:NSPLIT],
                        start=True, stop=False)
                    nc.tensor.matmul(
                        out=p2[:], lhsT=w2_t[:, 128:256], rhs=h1[:, NSPLIT:],
                        start=False, stop=True)
                    h2 = lp.tile([128, NSPLIT], mdt, tag="h2")
                    nc.scalar.activation(
                        out=h2[:], in_=p2[:],
                        func=mybir.ActivationFunctionType.Relu,
                        bias=b2_t[:, 0:1])
                    p3 = pm.tile([1, NSPLIT], f32, tag="p3")
                    nc.tensor.matmul(
                        out=p3[:], lhsT=w3_t[:, 0:1], rhs=h2[:],
                        start=True, stop=True)
                    osb = lp.tile([1, NSPLIT], f32, tag="osb")
                    nc.scalar.activation(
                        out=osb[:], in_=p3[:],
                        func=mybir.ActivationFunctionType.Sigmoid,
                        bias=b3_t[:, 0:1])
                    base = ch * CH + s * NSPLIT
                    nc.sync.dma_start(
                        out=out2d[0:1, base:base + NSPLIT], in_=osb[:])

    nc.compile()
    return nc


# ---------------------------------------------------------------- host prep
def _np_dt(name):
    if name == "f32":
        return np.float32
    import ml_dtypes
    return ml_dtypes.bfloat16


def prepare_in_maps(v_feat, W1, b1, W2, b2, W3, b3, node_ids):
    gnp = _np_dt(GATHER_DT)
    mnp = _np_dt(MLP_DT)

    vfeat_h = np.ascontiguousarray(np.asarray(v_feat, np.float32)).astype(gnp)
    w1_h = np.asarray(W1, np.float32).astype(mnp)                     # [128,256]
    b1_h = np.ascontiguousarray(np.asarray(b1, np.float32).reshape(2, 128).T)
    w2_h = np.concatenate(
        [np.asarray(W2, np.float32)[0:128, :], np.asarray(W2, np.float32)[128:256, :]],
        axis=1).astype(mnp)                                            # [128,256]
    b2_h = np.asarray(b2, np.float32).reshape(128, 1)
    w3_h = np.asarray(W3, np.float32).astype(mnp)                      # [128,1]
    b3_h = np.asarray(b3, np.float32).reshape(1, 1)

    nid = np.asarray(node_ids).astype(np.int32)                        # [E]

    in_maps = []
    for c in range(NCORES):
        # hedge h_local = ch*896 + b*128 + p  (p = partition)
        # idx layout: [p, ch*112 + b*16 + m]
        hl = (np.arange(NCHUNK)[:, None, None] * CH
              + np.arange(CHUNK_BLKS)[None, :, None] * 128
              + np.arange(128)[None, None, :])                         # [ch,b,p]
        hglob = c * H_CORE + np.minimum(hl, H_CORE - 1)                # clamp pad
        e = hglob[..., None] * G + np.arange(G)                        # [ch,b,p,m]
        idx_core = nid[e]                                              # [ch,b,p,m]
        idx_core = np.ascontiguousarray(
            idx_core.transpose(2, 0, 1, 3).reshape(128, BLKS * G))     # [p, 784]
        in_maps.append({
            "ident": np.eye(128, dtype=mnp),
            "vfeat": vfeat_h,
            "idx": idx_core,
            "w1": w1_h, "b1": b1_h,
            "w2": w2_h, "b2": b2_h,
            "w3": w3_h, "b3": b3_h,
        })
    return in_maps


def assemble_output(results):
    """results: list (per core) of {'out': [HPAD] f32} -> [NH, 1] f32."""
    outs = []
    for c in range(NCORES):
        o = np.asarray(results[c]["out"], np.float32).reshape(HPAD)
        # device layout: out[ch*896 + b*128 + p] = hedge (ch*896 + b*128 + p)
        outs.append(o[:H_CORE])
    return np.concatenate(outs).reshape(NH, 1)


# ---------------------------------------------------------------- entry
_CACHED_NC = None
LAST_RESULTS = None


def _ensure_ntff_hook():
    """The image's antenv lacks axon_hooks; if tracing is ever requested
    (e.g. BASS_TRACE in the environment), bass_utils would ImportError.
    Provide a stub so the run degrades gracefully instead of crashing."""
    import sys
    import types
    try:
        import antenv.axon_hooks  # noqa: F401
        return
    except ImportError:
        pass
    try:
        hook = None
        try:
            from trn_agent_boot.trn_boot import _ntff_profile_via_ctypes
            hook = _ntff_profile_via_ctypes("/opt/axon/libaxon_pjrt.so")
        except Exception:
            hook = None
        mod = types.ModuleType("antenv.axon_hooks")
        mod._hook = hook
        mod.get_axon_ntff_profile_hook = lambda: mod._hook
        mod.set_axon_ntff_profile_hook = lambda h: setattr(mod, "_hook", h)
        import antenv
        antenv.axon_hooks = mod
        sys.modules["antenv.axon_hooks"] = mod
    except Exception:
        pass


def _numpy_fallback(v_feat, W1, b1, W2, b2, W3, b3, node_ids, segment_ids):
    """General (slow, host) path for non-uniform segments; never taken for
    the reference's setup_inputs, which always emits repeat(arange(NH), 16)."""
    v = np.asarray(v_feat, np.float32)
    feats = v[np.asarray(node_ids).astype(np.int64)]
    seg = np.asarray(segment_ids).astype(np.int64)
    mx = np.full((NH, D), -np.inf, np.float32)
    mn = np.full((NH, D), np.inf, np.float32)
    np.maximum.at(mx, seg, feats)
    np.minimum.at(mn, seg, feats)
    emb = mx - mn
    h = np.maximum(emb @ np.asarray(W1, np.float32) + np.asarray(b1, np.float32), 0)
    h = np.maximum(h @ np.asarray(W2, np.float32) + np.asarray(b2, np.float32), 0)
    z = h @ np.asarray(W3, np.float32) + np.asarray(b3, np.float32)
    return (1.0 / (1.0 + np.exp(-z))).astype(np.float32)


def kernel(v_feat, W1, b1, W2, b2, W3, b3, node_ids, segment_ids):
    global _CACHED_NC, LAST_RESULTS

    seg = np.asarray(segment_ids)
    if seg.shape != (E,) or not np.array_equal(
            seg[::G], np.arange(NH, dtype=seg.dtype)) or not np.array_equal(
            seg, np.repeat(seg[::G], G)):
        return _numpy_fallback(v_feat, W1, b1, W2, b2, W3, b3,
                               node_ids, segment_ids)

    in_maps = prepare_in_maps(v_feat, W1, b1, W2, b2, W3, b3, node_ids)

    _ensure_ntff_hook()
    if _CACHED_NC is None:
        _CACHED_NC = build_module()
    nc = _CACHED_NC

    res = bass_utils.run_bass_kernel_spmd(
        nc, in_maps, core_ids=list(range(NCORES)))
    LAST_RESULTS = res
    return assemble_output(res.results)

